# revision 17
# baseline (speedup 1.0000x reference)
"""MoE (8 experts, top-2, shared expert) Trainium2 kernel.

Strategy (expert-parallel, per sharding hint):
  - Host computes routing (sigmoid gate -> top-2 -> stable sort by expert),
    exactly mirroring the jax reference in fp32 numpy, and plays the role of
    the all-to-all: expert e's tokens (scaled by router scores, bf16,
    zero-padded to capacity C) go to core e. The shared expert is
    data-parallel: core i gets tokens [i*256, (i+1)*256).
  - Device phases (tokens on the moving free dim, no on-chip transposes):
        shared-up   gT = su.T.T @ xsT   (fp16, fp32 accum) -> relu^2 fp16
        routed-up   hT = wu.T.T @ xrT   (bf16, fp32 accum) -> relu^2 bf16
        routed-down yrT = wd.T.T @ hT   (bf16) -> bf16 out
        shared-down ysT = sd.T.T @ gT   (fp16) -> fp16 out
    fp16 (10-bit mantissa) gives ~tf32 precision at half the f32 DMA bytes.
  - Host scatters per-expert outputs back to token order, sums top-2 + shared.

Performance structure (from NTFF trace analysis of the 85us baseline):
  - DRAM layouts are consumption-ordered: each weight matrix is packed as
    m-major slabs of k-tiles, so the PE's m-outer/k-inner loop consumes a
    linear DMA prefix and never waits for data beyond the next slab.
  - All DMAs ride ONE queue (sync) in PE-consumption order: a single queue
    issues packets FIFO across all 16 DMA engines at the full ~420GB/s
    aggregate, so arrival order == trigger order and no explicit deps are
    needed. (Splitting across the two hwdge queues fair-shares packets,
    destroying both ordering and per-stream bandwidth.)
  - Capacity factor 1.0: C = 512 = T*TOPK/E. One 512-wide f32 PSUM tile
    per (m,k) fills a bank exactly (pools are bank-granular, 6+2 = 8
    banks), every matmul is 512 or 256 columns (ldweights fully hidden),
    and xr k-tile SBUF offsets stay 128B-aligned. The per-expert overflow
    beyond C (44 of 4096 slots for this routing) is computed exactly on
    the host, which already runs the router/dispatch/combine.
  - Phases start with the shared expert (smallest startup gate: xs + the
    first su slab, ~0.8MB) and end with shared-down (the final store is a
    single 64KB slab). Note: the core's DVFS state is exogenous — the same
    binary measures ~82us (ldweights 97ns, 2.4GHz) or ~98us (116ns, 2.0GHz)
    run-to-run, with identical throttle counters.
  - Epilogues run on DVE (ACT is several times slower on plain relu/copy);
    Bacc.compile() legalizes sync-wait budgets.

Self-contained: hardcodes shapes from the problem spec.
"""

import numpy as np
import ml_dtypes
from contextlib import ExitStack

T = 2048          # tokens (BS*SLEN)
DIM = 1024
E = 8             # experts == cores
TOPK = 2
HID = 1408
NCORES = 8
S = T // NCORES   # shared-expert tokens per core

KD = DIM // 128   # 8  k-tiles contracting over DIM
MH = HID // 128   # 11 m-tiles over hidden
MD = DIM // 128   # 8  m-tiles over model dim

TRACE = False
TRACE_CORES = None
TRACE_DIR = None
LAST_RESULT = None   # BassKernelResults of the last run (for test harness)

NWARM = 11           # HAM warmup dummy matmuls (FD=512, cold ~427ns each)
NBRIDGE = 0          # redundant shared-tile recomputes at the s->r crossing

_PROG_CACHE = {}

bf16 = ml_dtypes.bfloat16


def _build_program(C):
    import concourse.tile as tile
    import concourse.mybir as mybir
    from concourse import bacc

    dt = mybir.dt
    # Bacc (not raw Bass): its compile() pass moves matmul waits onto
    # ldweights and splits over-budget sync waits into event semaphores —
    # without it walrus rejects instructions with >1 wait.
    nc = bacc.Bacc("TRN2", target_bir_lowering=False)

    assert C == 512
    SUB = KD * S           # su slab base inside sT
    WUB = KD * C           # wu slab base inside rT
    SW = SUB + MH * KD * 128    # fp16 shared-up stream width
    RW = WUB + MH * KD * 128    # bf16 routed-up stream width
    DW = MD * MH * 128          # down-proj slab stream width (wd / sd)
    YW = C + S                  # out: [yr bf16 | ys f32-as-fp16 slots]

    sT = nc.declare_dram_parameter("sT", [128, SW], dt.float16, isOutput=False)
    rT = nc.declare_dram_parameter("rT", [128, RW], dt.bfloat16, isOutput=False)
    dT = nc.declare_dram_parameter("dT", [128, DW], dt.bfloat16, isOutput=False)
    eT = nc.declare_dram_parameter("eT", [128, DW], dt.float16, isOutput=False)
    # Single output param: the drain's wait list covers PE + DVE + every
    # used DMA queue and holds at most 6 entries.
    yT = nc.declare_dram_parameter("yT", [128, MD, YW], dt.bfloat16,
                                   isOutput=True)

    with ExitStack() as ctx:
        tc = ctx.enter_context(tile.TileContext(nc))
        wpool = ctx.enter_context(tc.tile_pool(name="w", bufs=1))
        # PSUM pool bufs are bank-granular (2KB each); a 512-f32 tile
        # fills a bank exactly: psA 6 + psS 2 = all 8 banks.
        psA = ctx.enter_context(tc.tile_pool(name="psA", bufs=6, space="PSUM"))
        psS = ctx.enter_context(tc.tile_pool(name="psS", bufs=2, space="PSUM"))
        hpool = opool = wpool

        s_t = wpool.tile([128, SW], dt.float16, tag="s", name="s")
        r_t = wpool.tile([128, RW], dt.bfloat16, tag="r", name="r")
        d_t = wpool.tile([128, DW], dt.bfloat16, tag="d", name="d")
        e_t = wpool.tile([128, DW], dt.float16, tag="e", name="e")

        # --- HAM warmup: the PE clock-gate defaults to K=4/8 (1.2GHz) and
        # only reaches 2.4GHz after ~3.4us of sustained PE activity.  The
        # program body starts ~7.4us in (runtime handshake + program load +
        # tile barriers) but the first input chunk only lands ~11.5us in,
        # so the PE would otherwise idle AND start cold.  Dummy FD=512
        # matmuls on a zeroed tile fill that window: they warm the clock
        # and cost nothing (results go to rotating psA banks, never read;
        # the real accumulation groups reset has_written via start=True).
        warm = wpool.tile([128, 512], dt.bfloat16, tag="warm", name="warm")
        nc.vector.memset(warm[:], 0)
        for _ in range(NWARM):
            pw = psA.tile([128, 512], dt.float32, tag="psA", name="psA")
            nc.tensor.matmul(pw[:], warm[:, :128], warm[:], start=True,
                             stop=True)

        # ONE queue (sync), chunks in exact PE-consumption order: a single
        # queue issues packets FIFO across all 16 DMA engines at the full
        # ~380-420GB/s aggregate, so arrival order == trigger order. (Two
        # engine-queues fair-share packets instead, which destroys both
        # ordering and per-stream bandwidth — measured 98us that way.)
        # The supply curve pins the schedule: ~4.7MB must land before the
        # first routed m-tile can run, which at ~0.4GB/ms is ~t=21-22us.
        # All 11 shared m-tiles run first (their per-slab chunks land just
        # ahead of a warm PE), ending ~21.5us, so the crossing costs <1us
        # and never re-throttles the HAM (needs ~3.4us of idle).  Each
        # DMA instruction costs ~0.25-0.3us in the walrus teardown storm,
        # so late chunks are coarse.
        def sc(a, b):
            nc.sync.dma_start(s_t[:, a:b], sT[:, a:b])

        def rc(a, b):
            nc.sync.dma_start(r_t[:, a:b], rT[:, a:b])

        sc(0, SUB + 1024)                      # xs + su[m0]
        for m in range(1, 6):                  # su[m1..m5] per-slab
            sc(SUB + m * 1024, SUB + (m + 1) * 1024)
        sc(SUB + 6 * 1024, SUB + 8 * 1024)     # su[m6,m7]
        sc(SUB + 8 * 1024, SW)                 # su[m8..m10]
        rc(0, 8 * C)                           # xr[k0..7]
        rc(WUB, WUB + 2 * 1024)                # wu[m0,m1]
        rc(WUB + 2 * 1024, WUB + 6 * 1024)     # wu[m2..m5]
        rc(WUB + 6 * 1024, RW)                 # wu[m6..m10]
        # wd/sd have ~14us of arrival slack. NOTE: keep these as sliced
        # half-tile chunks — full-tile dma_start(d_t[:, :], dT[:, :]) works
        # on the traced path but silently corrupts on the untraced
        # bass2jax/pjrt path (measured rel_err 4e7).
        for a, b in ((0, DW // 2), (DW // 2, DW)):
            nc.sync.dma_start(d_t[:, a:b], dT[:, a:b])
        for a, b in ((0, DW // 2), (DW // 2, DW)):
            nc.sync.dma_start(e_t[:, a:b], eT[:, a:b])

        def xs(k):
            return s_t[:, k * S:(k + 1) * S]

        def su(k, m):
            o = SUB + m * KD * 128 + k * 128
            return s_t[:, o:o + 128]

        def xr(k, c0, cw):
            o = k * C + c0
            return r_t[:, o:o + cw]

        def wu(k, m):
            o = WUB + m * KD * 128 + k * 128
            return r_t[:, o:o + 128]

        def wd(k, m):
            o = m * MH * 128 + k * 128
            return d_t[:, o:o + 128]

        def sd(k, m):
            o = m * MH * 128 + k * 128
            return e_t[:, o:o + 128]

        # Epilogues all on DVE (ACT pays a LUT-table load per op and is
        # several times slower on plain relu/copy tiles).
        # --- up-projs: all 11 shared m-tiles, then all 11 routed ---
        g_t = hpool.tile([128, MH, S], dt.float16, tag="g", name="g")
        h_t = hpool.tile([128, MH, C], dt.bfloat16, tag="h", name="h")

        for m in range(MH):
            ps = psS.tile([128, S], dt.float32, tag="psS", name="psS")
            for k in range(KD):
                nc.tensor.matmul(ps[:], su(k, m), xs(k),
                                 start=(k == 0), stop=(k == KD - 1))
            v = g_t[:, m, :]
            nc.vector.tensor_relu(v, ps[:])
            nc.vector.tensor_mul(v, v, v)

        # Bridge over the DMA supply crossing: on slow-HBM cores the first
        # routed tile's data (cumulative ~4.7MB) lands ~2-3us after the
        # shared phase drains.  Recomputing a couple of shared tiles from
        # already-resident SBUF keeps the PE busy (HAM stays warm) at zero
        # DMA cost; results go to rotating psS banks and are never read.
        for j in range(NBRIDGE):
            ps = psS.tile([128, S], dt.float32, tag="psS", name="psS")
            for k in range(KD):
                nc.tensor.matmul(ps[:], su(k, j), xs(k),
                                 start=(k == 0), stop=(k == KD - 1))

        for m in range(MH):
            pa = psA.tile([128, 512], dt.float32, tag="psA", name="psA")
            for k in range(KD):
                nc.tensor.matmul(pa[:], wu(k, m), xr(k, 0, 512),
                                 start=(k == 0), stop=(k == KD - 1))
            v = h_t[:, m, :]
            nc.vector.tensor_relu(v, pa[:])
            nc.vector.tensor_mul(v, v, v)

        # --- routed down-proj -> staged yr stores on the sync queue ---
        ybr = opool.tile([128, MD, C], dt.bfloat16, tag="ybr", name="ybr")
        for m in range(MD):
            pa = psA.tile([128, 512], dt.float32, tag="psA", name="psA")
            for k in range(MH):
                nc.tensor.matmul(pa[:], wd(k, m), h_t[:, k, :],
                                 start=(k == 0), stop=(k == MH - 1))
            nc.vector.tensor_copy(ybr[:, m, :], pa[:])
            if m in (4, MD - 1):
                m0 = {4: 0, MD - 1: 5}[m]
                nc.sync.dma_start(yT[:, m0:m + 1, :C], ybr[:, m0:m + 1, :])

        # --- shared down-proj -> staged ys stores on the scalar queue;
        # the final store is a single small slab ---
        ybs = opool.tile([128, MD, S], dt.float16, tag="ybs", name="ybs")
        for m in range(MD):
            ps = psS.tile([128, S], dt.float32, tag="psS", name="psS")
            for k in range(MH):
                nc.tensor.matmul(ps[:], sd(k, m), g_t[:, k, :],
                                 start=(k == 0), stop=(k == MH - 1))
            nc.vector.tensor_copy(ybs[:, m, :], ps[:])
            if m in (3, 6, MD - 1):
                m0 = {3: 0, 6: 4, MD - 1: 7}[m]
                nc.sync.dma_start(
                    yT[:, m0:m + 1, C:].bitcast(dt.float16),
                    ybs[:, m0:m + 1, :])

    nc.compile()
    return nc


def _route(x, gate_w, expert_bias):
    """Exact numpy mirror of the reference TopKRouter + dispatch."""
    xf = x.reshape(-1, DIM).astype(np.float32)
    logits = xf @ gate_w.T.astype(np.float32)
    scores = 1.0 / (1.0 + np.exp(-logits.astype(np.float32)))
    biased = scores + expert_bias[None, :].astype(np.float32)
    # top-2, ties -> lower index (matches jax.lax.top_k)
    sel = np.argsort(-biased, axis=-1, kind="stable")[:, :TOPK]
    top_scores = np.take_along_axis(scores, sel, axis=-1)
    flat_sel = sel.reshape(-1)
    counts = np.bincount(flat_sel, minlength=E)
    order = np.argsort(flat_sel, kind="stable")
    scores_sorted = top_scores.reshape(-1)[order]
    token_ids = order // TOPK
    return xf, counts, order, token_ids, scores_sorted


def _kchunk(mat, width):
    """(n_k*128, width) row-major -> (128, n_k, width)."""
    return mat.reshape(-1, 128, width).transpose(1, 0, 2)


def _slabs(mat, n_m):
    """(n_k*128, n_m*128) weight matrix -> (128, n_m*n_k*128) m-major slabs
    of k-tiles, matching the device-side m-outer/k-inner consumption."""
    n_k = mat.shape[0] // 128
    kc = _kchunk(mat, mat.shape[1])                  # (128, n_k, n_m*128)
    return np.ascontiguousarray(
        kc.reshape(128, n_k, n_m, 128).transpose(0, 2, 1, 3).reshape(128, -1))


def kernel(x, gate_w, expert_bias, w_up, w_down, shared_w_up, shared_w_down):
    global LAST_RESULT
    from concourse.bass_utils import run_bass_kernel_spmd

    xf, counts, order, token_ids, scores_sorted = _route(x, gate_w, expert_bias)

    C = 512
    starts = np.zeros(E + 1, np.int64)
    np.cumsum(counts, out=starts[1:])

    # dispatch: routed_in rows grouped by expert, scaled by router score
    routed_in = (xf[token_ids] * scores_sorted[:, None]).astype(np.float32)
    routed_in = routed_in.astype(bf16)

    su_pack = _slabs(shared_w_up.T.astype(np.float16), MH)    # (128, 11264)
    sd_pack = _slabs(shared_w_down.T.astype(np.float16), MD)  # (128, 11264)
    in_maps = []
    for e in range(NCORES):
        seg = routed_in[starts[e]:starts[e] + min(int(counts[e]), C)]
        xrm = np.zeros((C, DIM), bf16)
        xrm[:seg.shape[0]] = seg
        xs_pack = _kchunk(
            xf[e * S:(e + 1) * S].T.astype(np.float16), S).reshape(128, -1)
        in_maps.append({
            "sT": np.ascontiguousarray(
                np.concatenate([xs_pack, su_pack], axis=1)),
            "rT": np.ascontiguousarray(np.concatenate(
                [_kchunk(xrm.T, C).reshape(128, -1),
                 _slabs(w_up[e].astype(bf16).T, MH)], axis=1)),
            "dT": _slabs(w_down[e].astype(bf16).T, MD),
            "eT": sd_pack,
        })

    key = (C, NWARM, NBRIDGE)
    if key not in _PROG_CACHE:
        _PROG_CACHE[key] = _build_program(C)
    nc = _PROG_CACHE[key]

    res = run_bass_kernel_spmd(
        nc, in_maps, list(range(NCORES)),
        trace=TRACE,
        trace_cores=TRACE_CORES,
        tmpdir=TRACE_DIR,
    )
    LAST_RESULT = res

    # --- combine (host): scatter per-expert outputs back to token order;
    # overflow slots beyond capacity get the exact fp32 expert MLP here ---
    routed_sorted = np.empty((T * TOPK, DIM), np.float32)
    for e in range(NCORES):
        arr = np.asarray(res.results[e]["yT"])             # (128, MD, C+S)
        yr = arr[:, :, :C].transpose(1, 0, 2).reshape(DIM, C).T
        n_dev = min(int(counts[e]), C)
        routed_sorted[starts[e]:starts[e] + n_dev] = \
            yr[:n_dev].astype(np.float32)
        if counts[e] > C:
            xo = routed_in[starts[e] + C:starts[e + 1]].astype(np.float32)
            ho = np.square(np.maximum(xo @ w_up[e].T.astype(np.float32), 0)
                           .astype(bf16).astype(np.float32)).astype(bf16)
            yo = ho.astype(np.float32) @ w_down[e].T.astype(np.float32)
            routed_sorted[starts[e] + C:starts[e + 1]] = \
                yo.astype(bf16).astype(np.float32)
    combined = np.empty((T * TOPK, DIM), np.float32)
    combined[order] = routed_sorted
    out = combined.reshape(T, TOPK, DIM).sum(axis=1)

    for e in range(NCORES):
        arr = np.asarray(res.results[e]["yT"])
        ys = np.ascontiguousarray(arr[:, :, C:]).view(np.float16)  # (128,MD,S)
        out[e * S:(e + 1) * S] += ys.transpose(1, 0, 2).reshape(DIM, S).T

    return out.reshape(1, T, DIM).astype(np.float32)



# revision 18
# speedup vs baseline: 1.0445x; 1.0445x over previous
"""MoE (8 experts, top-2, shared expert) Trainium2 kernel.

Strategy (expert-parallel, per sharding hint):
  - Host computes routing (sigmoid gate -> top-2 -> stable sort by expert),
    exactly mirroring the jax reference in fp32 numpy, and plays the role of
    the all-to-all: expert e's tokens (scaled by router scores, bf16,
    zero-padded to capacity C) go to core e. The shared expert is
    data-parallel: core i gets tokens [i*256, (i+1)*256).
  - Device phases (tokens on the moving free dim, no on-chip transposes):
        shared-up   gT = su.T.T @ xsT   (fp16, fp32 accum) -> relu^2 fp16
        routed-up   hT = wu.T.T @ xrT   (bf16, fp32 accum) -> relu^2 bf16
        routed-down yrT = wd.T.T @ hT   (bf16) -> bf16 out
        shared-down ysT = sd.T.T @ gT   (fp16) -> fp16 out
    fp16 (10-bit mantissa) gives ~tf32 precision at half the f32 DMA bytes.
  - Host scatters per-expert outputs back to token order, sums top-2 + shared.

Performance structure (from NTFF trace analysis of the 85us baseline):
  - DRAM layouts are consumption-ordered: each weight matrix is packed as
    m-major slabs of k-tiles, so the PE's m-outer/k-inner loop consumes a
    linear DMA prefix and never waits for data beyond the next slab.
  - All DMAs ride ONE queue (sync) in PE-consumption order: a single queue
    issues packets FIFO across all 16 DMA engines at the full ~420GB/s
    aggregate, so arrival order == trigger order and no explicit deps are
    needed. (Splitting across the two hwdge queues fair-shares packets,
    destroying both ordering and per-stream bandwidth.)
  - Capacity factor 1.0: C = 512 = T*TOPK/E. One 512-wide f32 PSUM tile
    per (m,k) fills a bank exactly (pools are bank-granular, 6+2 = 8
    banks), every matmul is 512 or 256 columns (ldweights fully hidden),
    and xr k-tile SBUF offsets stay 128B-aligned. The per-expert overflow
    beyond C (44 of 4096 slots for this routing) is computed exactly on
    the host, which already runs the router/dispatch/combine.
  - Phases start with the shared expert (smallest startup gate: xs + the
    first su slab, ~0.8MB) and end with shared-down (the final store is a
    single 64KB slab). Note: the core's DVFS state is exogenous — the same
    binary measures ~82us (ldweights 97ns, 2.4GHz) or ~98us (116ns, 2.0GHz)
    run-to-run, with identical throttle counters.
  - Epilogues run on DVE (ACT is several times slower on plain relu/copy);
    Bacc.compile() legalizes sync-wait budgets.

Self-contained: hardcodes shapes from the problem spec.
"""

import numpy as np
import ml_dtypes
from contextlib import ExitStack

T = 2048          # tokens (BS*SLEN)
DIM = 1024
E = 8             # experts == cores
TOPK = 2
HID = 1408
NCORES = 8
S = T // NCORES   # shared-expert tokens per core

KD = DIM // 128   # 8  k-tiles contracting over DIM
MH = HID // 128   # 11 m-tiles over hidden
MD = DIM // 128   # 8  m-tiles over model dim

TRACE = False
TRACE_CORES = None
TRACE_DIR = None
LAST_RESULT = None   # BassKernelResults of the last run (for test harness)

NWARM = 11           # HAM warmup dummy matmuls (FD=512, cold ~427ns each)
NBRIDGE = 0          # redundant shared-tile recomputes at the s->r crossing

_PROG_CACHE = {}

bf16 = ml_dtypes.bfloat16


def _build_program(C):
    import concourse.tile as tile
    import concourse.mybir as mybir
    from concourse import bacc

    dt = mybir.dt
    # Bacc (not raw Bass): its compile() pass moves matmul waits onto
    # ldweights and splits over-budget sync waits into event semaphores —
    # without it walrus rejects instructions with >1 wait.
    nc = bacc.Bacc("TRN2", target_bir_lowering=False)

    assert C == 512
    SUB = KD * S           # su slab base inside sT
    WUB = KD * C           # wu slab base inside rT
    SW = SUB + MH * KD * 128    # fp16 shared-up stream width
    RW = WUB + MH * KD * 128    # bf16 routed-up stream width
    DW = MD * MH * 128          # down-proj slab stream width (wd / sd)
    YW = C + S                  # out: [yr bf16 | ys f32-as-fp16 slots]

    sT = nc.declare_dram_parameter("sT", [128, SW], dt.float16, isOutput=False)
    rT = nc.declare_dram_parameter("rT", [128, RW], dt.bfloat16, isOutput=False)
    dT = nc.declare_dram_parameter("dT", [128, DW], dt.bfloat16, isOutput=False)
    eT = nc.declare_dram_parameter("eT", [128, DW], dt.float16, isOutput=False)
    # Single output param: the drain's wait list covers PE + DVE + every
    # used DMA queue and holds at most 6 entries.
    yT = nc.declare_dram_parameter("yT", [128, MD, YW], dt.bfloat16,
                                   isOutput=True)

    with ExitStack() as ctx:
        tc = ctx.enter_context(tile.TileContext(nc))
        wpool = ctx.enter_context(tc.tile_pool(name="w", bufs=1))
        # PSUM pool bufs are bank-granular (2KB each); a 512-f32 tile
        # fills a bank exactly: psA 6 + psS 2 = all 8 banks.
        psA = ctx.enter_context(tc.tile_pool(name="psA", bufs=6, space="PSUM"))
        psS = ctx.enter_context(tc.tile_pool(name="psS", bufs=2, space="PSUM"))
        hpool = opool = wpool

        s_t = wpool.tile([128, SW], dt.float16, tag="s", name="s")
        r_t = wpool.tile([128, RW], dt.bfloat16, tag="r", name="r")
        d_t = wpool.tile([128, DW], dt.bfloat16, tag="d", name="d")
        e_t = wpool.tile([128, DW], dt.float16, tag="e", name="e")

        # --- HAM warmup: the PE clock-gate defaults to K=4/8 (1.2GHz) and
        # only reaches 2.4GHz after ~3.4us of sustained PE activity.  The
        # program body starts ~7.4us in (runtime handshake + program load +
        # tile barriers) but the first input chunk only lands ~11.5us in,
        # so the PE would otherwise idle AND start cold.  Dummy FD=512
        # matmuls on a zeroed tile fill that window: they warm the clock
        # and cost nothing (results go to rotating psA banks, never read;
        # the real accumulation groups reset has_written via start=True).
        warm = wpool.tile([128, 512], dt.bfloat16, tag="warm", name="warm")
        nc.vector.memset(warm[:], 0)
        for _ in range(NWARM):
            pw = psA.tile([128, 512], dt.float32, tag="psA", name="psA")
            nc.tensor.matmul(pw[:], warm[:, :128], warm[:], start=True,
                             stop=True)

        # ONE queue (sync), chunks in exact PE-consumption order: a single
        # queue issues packets FIFO across all 16 DMA engines at the full
        # ~380-420GB/s aggregate, so arrival order == trigger order. (Two
        # engine-queues fair-share packets instead, which destroys both
        # ordering and per-stream bandwidth — measured 98us that way.)
        # The supply curve pins the schedule: ~4.7MB must land before the
        # first routed m-tile can run, which at ~0.4GB/ms is ~t=21-22us.
        # All 11 shared m-tiles run first (their per-slab chunks land just
        # ahead of a warm PE), ending ~21.5us, so the crossing costs <1us
        # and never re-throttles the HAM (needs ~3.4us of idle).  Each
        # DMA instruction costs ~0.25-0.3us in the walrus teardown storm,
        # so late chunks are coarse.
        def sc(a, b):
            nc.sync.dma_start(s_t[:, a:b], sT[:, a:b])

        def rc(a, b):
            nc.sync.dma_start(r_t[:, a:b], rT[:, a:b])

        sc(0, SUB + 1024)                      # xs + su[m0]
        for m in range(1, 6):                  # su[m1..m5] per-slab
            sc(SUB + m * 1024, SUB + (m + 1) * 1024)
        sc(SUB + 6 * 1024, SUB + 8 * 1024)     # su[m6,m7]
        sc(SUB + 8 * 1024, SW)                 # su[m8..m10]
        rc(0, 8 * C)                           # xr[k0..7]
        rc(WUB, WUB + 1024)                    # wu[m0]
        rc(WUB + 1024, WUB + 2 * 1024)         # wu[m1]
        rc(WUB + 2 * 1024, WUB + 4 * 1024)     # wu[m2,m3]
        rc(WUB + 4 * 1024, WUB + 6 * 1024)     # wu[m4,m5]
        rc(WUB + 6 * 1024, RW)                 # wu[m6..m10]
        # wd/sd have ~14us of arrival slack. NOTE: keep these as sliced
        # half-tile chunks — full-tile dma_start(d_t[:, :], dT[:, :]) works
        # on the traced path but silently corrupts on the untraced
        # bass2jax/pjrt path (measured rel_err 4e7).
        for a, b in ((0, DW // 2), (DW // 2, DW)):
            nc.sync.dma_start(d_t[:, a:b], dT[:, a:b])
        for a, b in ((0, DW // 2), (DW // 2, DW)):
            nc.sync.dma_start(e_t[:, a:b], eT[:, a:b])

        def xs(k):
            return s_t[:, k * S:(k + 1) * S]

        def su(k, m):
            o = SUB + m * KD * 128 + k * 128
            return s_t[:, o:o + 128]

        def xr(k, c0, cw):
            o = k * C + c0
            return r_t[:, o:o + cw]

        def wu(k, m):
            o = WUB + m * KD * 128 + k * 128
            return r_t[:, o:o + 128]

        def wd(k, m):
            o = m * MH * 128 + k * 128
            return d_t[:, o:o + 128]

        def sd(k, m):
            o = m * MH * 128 + k * 128
            return e_t[:, o:o + 128]

        # Epilogues all on DVE (ACT pays a LUT-table load per op and is
        # several times slower on plain relu/copy tiles).
        # --- up-projs: all 11 shared m-tiles, then all 11 routed ---
        g_t = hpool.tile([128, MH, S], dt.float16, tag="g", name="g")
        h_t = hpool.tile([128, MH, C], dt.bfloat16, tag="h", name="h")

        for m in range(MH):
            ps = psS.tile([128, S], dt.float32, tag="psS", name="psS")
            for k in range(KD):
                nc.tensor.matmul(ps[:], su(k, m), xs(k),
                                 start=(k == 0), stop=(k == KD - 1))
            v = g_t[:, m, :]
            nc.vector.tensor_relu(v, ps[:])
            nc.vector.tensor_mul(v, v, v)

        # Bridge over the DMA supply crossing: on slow-HBM cores the first
        # routed tile's data (cumulative ~4.7MB) lands ~2-3us after the
        # shared phase drains.  Recomputing a couple of shared tiles from
        # already-resident SBUF keeps the PE busy (HAM stays warm) at zero
        # DMA cost; results go to rotating psS banks and are never read.
        for j in range(NBRIDGE):
            ps = psS.tile([128, S], dt.float32, tag="psS", name="psS")
            for k in range(KD):
                nc.tensor.matmul(ps[:], su(k, j), xs(k),
                                 start=(k == 0), stop=(k == KD - 1))

        for m in range(MH):
            pa = psA.tile([128, 512], dt.float32, tag="psA", name="psA")
            for k in range(KD):
                nc.tensor.matmul(pa[:], wu(k, m), xr(k, 0, 512),
                                 start=(k == 0), stop=(k == KD - 1))
            v = h_t[:, m, :]
            nc.vector.tensor_relu(v, pa[:])
            nc.vector.tensor_mul(v, v, v)

        # --- routed down-proj -> staged yr stores on the sync queue ---
        ybr = opool.tile([128, MD, C], dt.bfloat16, tag="ybr", name="ybr")
        for m in range(MD):
            pa = psA.tile([128, 512], dt.float32, tag="psA", name="psA")
            for k in range(MH):
                nc.tensor.matmul(pa[:], wd(k, m), h_t[:, k, :],
                                 start=(k == 0), stop=(k == MH - 1))
            nc.vector.tensor_copy(ybr[:, m, :], pa[:])
            if m in (4, MD - 1):
                m0 = {4: 0, MD - 1: 5}[m]
                nc.sync.dma_start(yT[:, m0:m + 1, :C], ybr[:, m0:m + 1, :])

        # --- shared down-proj -> staged ys stores on the scalar queue;
        # the final store is a single small slab ---
        ybs = opool.tile([128, MD, S], dt.float16, tag="ybs", name="ybs")
        for m in range(MD):
            ps = psS.tile([128, S], dt.float32, tag="psS", name="psS")
            for k in range(MH):
                nc.tensor.matmul(ps[:], sd(k, m), g_t[:, k, :],
                                 start=(k == 0), stop=(k == MH - 1))
            nc.vector.tensor_copy(ybs[:, m, :], ps[:])
            if m in (3, 6, MD - 1):
                m0 = {3: 0, 6: 4, MD - 1: 7}[m]
                nc.sync.dma_start(
                    yT[:, m0:m + 1, C:].bitcast(dt.float16),
                    ybs[:, m0:m + 1, :])

    nc.compile()
    return nc


def _route(x, gate_w, expert_bias):
    """Exact numpy mirror of the reference TopKRouter + dispatch."""
    xf = x.reshape(-1, DIM).astype(np.float32)
    logits = xf @ gate_w.T.astype(np.float32)
    scores = 1.0 / (1.0 + np.exp(-logits.astype(np.float32)))
    biased = scores + expert_bias[None, :].astype(np.float32)
    # top-2, ties -> lower index (matches jax.lax.top_k)
    sel = np.argsort(-biased, axis=-1, kind="stable")[:, :TOPK]
    top_scores = np.take_along_axis(scores, sel, axis=-1)
    flat_sel = sel.reshape(-1)
    counts = np.bincount(flat_sel, minlength=E)
    order = np.argsort(flat_sel, kind="stable")
    scores_sorted = top_scores.reshape(-1)[order]
    token_ids = order // TOPK
    return xf, counts, order, token_ids, scores_sorted


def _kchunk(mat, width):
    """(n_k*128, width) row-major -> (128, n_k, width)."""
    return mat.reshape(-1, 128, width).transpose(1, 0, 2)


def _slabs(mat, n_m):
    """(n_k*128, n_m*128) weight matrix -> (128, n_m*n_k*128) m-major slabs
    of k-tiles, matching the device-side m-outer/k-inner consumption."""
    n_k = mat.shape[0] // 128
    kc = _kchunk(mat, mat.shape[1])                  # (128, n_k, n_m*128)
    return np.ascontiguousarray(
        kc.reshape(128, n_k, n_m, 128).transpose(0, 2, 1, 3).reshape(128, -1))


def kernel(x, gate_w, expert_bias, w_up, w_down, shared_w_up, shared_w_down):
    global LAST_RESULT
    from concourse.bass_utils import run_bass_kernel_spmd

    xf, counts, order, token_ids, scores_sorted = _route(x, gate_w, expert_bias)

    C = 512
    starts = np.zeros(E + 1, np.int64)
    np.cumsum(counts, out=starts[1:])

    # dispatch: routed_in rows grouped by expert, scaled by router score
    routed_in = (xf[token_ids] * scores_sorted[:, None]).astype(np.float32)
    routed_in = routed_in.astype(bf16)

    su_pack = _slabs(shared_w_up.T.astype(np.float16), MH)    # (128, 11264)
    sd_pack = _slabs(shared_w_down.T.astype(np.float16), MD)  # (128, 11264)
    in_maps = []
    for e in range(NCORES):
        seg = routed_in[starts[e]:starts[e] + min(int(counts[e]), C)]
        xrm = np.zeros((C, DIM), bf16)
        xrm[:seg.shape[0]] = seg
        xs_pack = _kchunk(
            xf[e * S:(e + 1) * S].T.astype(np.float16), S).reshape(128, -1)
        in_maps.append({
            "sT": np.ascontiguousarray(
                np.concatenate([xs_pack, su_pack], axis=1)),
            "rT": np.ascontiguousarray(np.concatenate(
                [_kchunk(xrm.T, C).reshape(128, -1),
                 _slabs(w_up[e].astype(bf16).T, MH)], axis=1)),
            "dT": _slabs(w_down[e].astype(bf16).T, MD),
            "eT": sd_pack,
        })

    key = (C, NWARM, NBRIDGE)
    if key not in _PROG_CACHE:
        _PROG_CACHE[key] = _build_program(C)
    nc = _PROG_CACHE[key]

    res = run_bass_kernel_spmd(
        nc, in_maps, list(range(NCORES)),
        trace=TRACE,
        trace_cores=TRACE_CORES,
        tmpdir=TRACE_DIR,
    )
    LAST_RESULT = res

    # --- combine (host): scatter per-expert outputs back to token order;
    # overflow slots beyond capacity get the exact fp32 expert MLP here ---
    routed_sorted = np.empty((T * TOPK, DIM), np.float32)
    for e in range(NCORES):
        arr = np.asarray(res.results[e]["yT"])             # (128, MD, C+S)
        yr = arr[:, :, :C].transpose(1, 0, 2).reshape(DIM, C).T
        n_dev = min(int(counts[e]), C)
        routed_sorted[starts[e]:starts[e] + n_dev] = \
            yr[:n_dev].astype(np.float32)
        if counts[e] > C:
            xo = routed_in[starts[e] + C:starts[e + 1]].astype(np.float32)
            ho = np.square(np.maximum(xo @ w_up[e].T.astype(np.float32), 0)
                           .astype(bf16).astype(np.float32)).astype(bf16)
            yo = ho.astype(np.float32) @ w_down[e].T.astype(np.float32)
            routed_sorted[starts[e] + C:starts[e + 1]] = \
                yo.astype(bf16).astype(np.float32)
    combined = np.empty((T * TOPK, DIM), np.float32)
    combined[order] = routed_sorted
    out = combined.reshape(T, TOPK, DIM).sum(axis=1)

    for e in range(NCORES):
        arr = np.asarray(res.results[e]["yT"])
        ys = np.ascontiguousarray(arr[:, :, C:]).view(np.float16)  # (128,MD,S)
        out[e * S:(e + 1) * S] += ys.transpose(1, 0, 2).reshape(DIM, S).T

    return out.reshape(1, T, DIM).astype(np.float32)



# revision 19
# speedup vs baseline: 1.0490x; 1.0043x over previous
"""MoE (8 experts, top-2, shared expert) Trainium2 kernel.

Strategy (expert-parallel, per sharding hint):
  - Host computes routing (sigmoid gate -> top-2 -> stable sort by expert),
    exactly mirroring the jax reference in fp32 numpy, and plays the role of
    the all-to-all: expert e's tokens (scaled by router scores, bf16,
    zero-padded to capacity C) go to core e. The shared expert is
    data-parallel: core i gets tokens [i*256, (i+1)*256).
  - Device phases (tokens on the moving free dim, no on-chip transposes):
        shared-up   gT = su.T.T @ xsT   (fp16, fp32 accum) -> relu^2 fp16
        routed-up   hT = wu.T.T @ xrT   (bf16, fp32 accum) -> relu^2 bf16
        routed-down yrT = wd.T.T @ hT   (bf16) -> bf16 out
        shared-down ysT = sd.T.T @ gT   (fp16) -> fp16 out
    fp16 (10-bit mantissa) gives ~tf32 precision at half the f32 DMA bytes.
  - Host scatters per-expert outputs back to token order, sums top-2 + shared.

Performance structure (from NTFF trace analysis of the 85us baseline):
  - DRAM layouts are consumption-ordered: each weight matrix is packed as
    m-major slabs of k-tiles, so the PE's m-outer/k-inner loop consumes a
    linear DMA prefix and never waits for data beyond the next slab.
  - All DMAs ride ONE queue (sync) in PE-consumption order: a single queue
    issues packets FIFO across all 16 DMA engines at the full ~420GB/s
    aggregate, so arrival order == trigger order and no explicit deps are
    needed. (Splitting across the two hwdge queues fair-shares packets,
    destroying both ordering and per-stream bandwidth.)
  - Capacity factor 1.0: C = 512 = T*TOPK/E. One 512-wide f32 PSUM tile
    per (m,k) fills a bank exactly (pools are bank-granular, 6+2 = 8
    banks), every matmul is 512 or 256 columns (ldweights fully hidden),
    and xr k-tile SBUF offsets stay 128B-aligned. The per-expert overflow
    beyond C (44 of 4096 slots for this routing) is computed exactly on
    the host, which already runs the router/dispatch/combine.
  - Phases start with the shared expert (smallest startup gate: xs + the
    first su slab, ~0.8MB) and end with shared-down (the final store is a
    single 64KB slab). Note: the core's DVFS state is exogenous — the same
    binary measures ~82us (ldweights 97ns, 2.4GHz) or ~98us (116ns, 2.0GHz)
    run-to-run, with identical throttle counters.
  - Epilogues run on DVE (ACT is several times slower on plain relu/copy);
    Bacc.compile() legalizes sync-wait budgets.

Self-contained: hardcodes shapes from the problem spec.
"""

import numpy as np
import ml_dtypes
from contextlib import ExitStack

T = 2048          # tokens (BS*SLEN)
DIM = 1024
E = 8             # experts == cores
TOPK = 2
HID = 1408
NCORES = 8
S = T // NCORES   # shared-expert tokens per core

KD = DIM // 128   # 8  k-tiles contracting over DIM
MH = HID // 128   # 11 m-tiles over hidden
MD = DIM // 128   # 8  m-tiles over model dim

TRACE = False
TRACE_CORES = None
TRACE_DIR = None
LAST_RESULT = None   # BassKernelResults of the last run (for test harness)

NWARM = 9            # HAM warmup dummy matmuls (FD=512, cold ~427ns each)
NBRIDGE = 0          # redundant shared-tile recomputes at the s->r crossing

_PROG_CACHE = {}

bf16 = ml_dtypes.bfloat16


def _build_program(C):
    import concourse.tile as tile
    import concourse.mybir as mybir
    from concourse import bacc

    dt = mybir.dt
    # Bacc (not raw Bass): its compile() pass moves matmul waits onto
    # ldweights and splits over-budget sync waits into event semaphores —
    # without it walrus rejects instructions with >1 wait.
    nc = bacc.Bacc("TRN2", target_bir_lowering=False)

    assert C == 512
    SUB = KD * S           # su slab base inside sT
    WUB = KD * C           # wu slab base inside rT
    SW = SUB + MH * KD * 128    # fp16 shared-up stream width
    RW = WUB + MH * KD * 128    # bf16 routed-up stream width
    DW = MD * MH * 128          # down-proj slab stream width (wd / sd)
    YW = C + S                  # out: [yr bf16 | ys f32-as-fp16 slots]

    sT = nc.declare_dram_parameter("sT", [128, SW], dt.float16, isOutput=False)
    rT = nc.declare_dram_parameter("rT", [128, RW], dt.bfloat16, isOutput=False)
    dT = nc.declare_dram_parameter("dT", [128, DW], dt.bfloat16, isOutput=False)
    eT = nc.declare_dram_parameter("eT", [128, DW], dt.float16, isOutput=False)
    # Single output param: the drain's wait list covers PE + DVE + every
    # used DMA queue and holds at most 6 entries.
    yT = nc.declare_dram_parameter("yT", [128, MD, YW], dt.bfloat16,
                                   isOutput=True)

    with ExitStack() as ctx:
        tc = ctx.enter_context(tile.TileContext(nc))
        wpool = ctx.enter_context(tc.tile_pool(name="w", bufs=1))
        # PSUM pool bufs are bank-granular (2KB each); a 512-f32 tile
        # fills a bank exactly: psA 6 + psS 2 = all 8 banks.
        psA = ctx.enter_context(tc.tile_pool(name="psA", bufs=6, space="PSUM"))
        psS = ctx.enter_context(tc.tile_pool(name="psS", bufs=2, space="PSUM"))
        hpool = opool = wpool

        s_t = wpool.tile([128, SW], dt.float16, tag="s", name="s")
        r_t = wpool.tile([128, RW], dt.bfloat16, tag="r", name="r")
        d_t = wpool.tile([128, DW], dt.bfloat16, tag="d", name="d")
        e_t = wpool.tile([128, DW], dt.float16, tag="e", name="e")

        # --- HAM warmup: the PE clock-gate defaults to K=4/8 (1.2GHz) and
        # only reaches 2.4GHz after ~3.4us of sustained PE activity.  The
        # program body starts ~7.4us in (runtime handshake + program load +
        # tile barriers) but the first input chunk only lands ~11.5us in,
        # so the PE would otherwise idle AND start cold.  Dummy FD=512
        # matmuls on a zeroed tile fill that window: they warm the clock
        # and cost nothing (results go to rotating psA banks, never read;
        # the real accumulation groups reset has_written via start=True).
        warm = wpool.tile([128, 512], dt.bfloat16, tag="warm", name="warm")
        nc.vector.memset(warm[:], 0)
        for _ in range(NWARM):
            pw = psA.tile([128, 512], dt.float32, tag="psA", name="psA")
            nc.tensor.matmul(pw[:], warm[:, :128], warm[:], start=True,
                             stop=True)

        # ONE queue (sync), chunks in exact PE-consumption order: a single
        # queue issues packets FIFO across all 16 DMA engines at the full
        # ~380-420GB/s aggregate, so arrival order == trigger order. (Two
        # engine-queues fair-share packets instead, which destroys both
        # ordering and per-stream bandwidth — measured 98us that way.)
        # The supply curve pins the schedule: ~4.7MB must land before the
        # first routed m-tile can run, which at ~0.4GB/ms is ~t=21-22us.
        # All 11 shared m-tiles run first (their per-slab chunks land just
        # ahead of a warm PE), ending ~21.5us, so the crossing costs <1us
        # and never re-throttles the HAM (needs ~3.4us of idle).  Each
        # DMA instruction costs ~0.25-0.3us in the walrus teardown storm,
        # so late chunks are coarse.
        def sc(a, b):
            nc.sync.dma_start(s_t[:, a:b], sT[:, a:b])

        def rc(a, b):
            nc.sync.dma_start(r_t[:, a:b], rT[:, a:b])

        sc(0, SUB + 1024)                      # xs + su[m0]
        for m in range(1, 6):                  # su[m1..m5] per-slab
            sc(SUB + m * 1024, SUB + (m + 1) * 1024)
        sc(SUB + 6 * 1024, SUB + 8 * 1024)     # su[m6,m7]
        sc(SUB + 8 * 1024, SW)                 # su[m8..m10]
        rc(0, 8 * C)                           # xr[k0..7]
        rc(WUB, WUB + 1024)                    # wu[m0]
        rc(WUB + 1024, WUB + 2 * 1024)         # wu[m1]
        rc(WUB + 2 * 1024, WUB + 4 * 1024)     # wu[m2,m3]
        rc(WUB + 4 * 1024, WUB + 6 * 1024)     # wu[m4,m5]
        rc(WUB + 6 * 1024, RW)                 # wu[m6..m10]
        # wd/sd have ~14us of arrival slack. NOTE: keep these as sliced
        # half-tile chunks — full-tile dma_start(d_t[:, :], dT[:, :]) works
        # on the traced path but silently corrupts on the untraced
        # bass2jax/pjrt path (measured rel_err 4e7).
        for a, b in ((0, DW // 2), (DW // 2, DW)):
            nc.sync.dma_start(d_t[:, a:b], dT[:, a:b])
        for a, b in ((0, DW // 2), (DW // 2, DW)):
            nc.sync.dma_start(e_t[:, a:b], eT[:, a:b])

        def xs(k):
            return s_t[:, k * S:(k + 1) * S]

        def su(k, m):
            o = SUB + m * KD * 128 + k * 128
            return s_t[:, o:o + 128]

        def xr(k, c0, cw):
            o = k * C + c0
            return r_t[:, o:o + cw]

        def wu(k, m):
            o = WUB + m * KD * 128 + k * 128
            return r_t[:, o:o + 128]

        def wd(k, m):
            o = m * MH * 128 + k * 128
            return d_t[:, o:o + 128]

        def sd(k, m):
            o = m * MH * 128 + k * 128
            return e_t[:, o:o + 128]

        # Epilogues all on DVE (ACT pays a LUT-table load per op and is
        # several times slower on plain relu/copy tiles).
        # --- up-projs: all 11 shared m-tiles, then all 11 routed ---
        g_t = hpool.tile([128, MH, S], dt.float16, tag="g", name="g")
        h_t = hpool.tile([128, MH, C], dt.bfloat16, tag="h", name="h")

        for m in range(MH):
            ps = psS.tile([128, S], dt.float32, tag="psS", name="psS")
            for k in range(KD):
                nc.tensor.matmul(ps[:], su(k, m), xs(k),
                                 start=(k == 0), stop=(k == KD - 1))
            v = g_t[:, m, :]
            nc.vector.tensor_relu(v, ps[:])
            nc.vector.tensor_mul(v, v, v)

        # Bridge over the DMA supply crossing: on slow-HBM cores the first
        # routed tile's data (cumulative ~4.7MB) lands ~2-3us after the
        # shared phase drains.  Recomputing a couple of shared tiles from
        # already-resident SBUF keeps the PE busy (HAM stays warm) at zero
        # DMA cost; results go to rotating psS banks and are never read.
        for j in range(NBRIDGE):
            ps = psS.tile([128, S], dt.float32, tag="psS", name="psS")
            for k in range(KD):
                nc.tensor.matmul(ps[:], su(k, j), xs(k),
                                 start=(k == 0), stop=(k == KD - 1))

        for m in range(MH):
            pa = psA.tile([128, 512], dt.float32, tag="psA", name="psA")
            for k in range(KD):
                nc.tensor.matmul(pa[:], wu(k, m), xr(k, 0, 512),
                                 start=(k == 0), stop=(k == KD - 1))
            v = h_t[:, m, :]
            nc.vector.tensor_relu(v, pa[:])
            nc.vector.tensor_mul(v, v, v)

        # --- routed down-proj -> staged yr stores on the sync queue ---
        ybr = opool.tile([128, MD, C], dt.bfloat16, tag="ybr", name="ybr")
        for m in range(MD):
            pa = psA.tile([128, 512], dt.float32, tag="psA", name="psA")
            for k in range(MH):
                nc.tensor.matmul(pa[:], wd(k, m), h_t[:, k, :],
                                 start=(k == 0), stop=(k == MH - 1))
            nc.vector.tensor_copy(ybr[:, m, :], pa[:])
            if m in (4, MD - 1):
                m0 = {4: 0, MD - 1: 5}[m]
                nc.sync.dma_start(yT[:, m0:m + 1, :C], ybr[:, m0:m + 1, :])

        # --- shared down-proj -> staged ys stores on the scalar queue;
        # the final store is a single small slab ---
        ybs = opool.tile([128, MD, S], dt.float16, tag="ybs", name="ybs")
        for m in range(MD):
            ps = psS.tile([128, S], dt.float32, tag="psS", name="psS")
            for k in range(MH):
                nc.tensor.matmul(ps[:], sd(k, m), g_t[:, k, :],
                                 start=(k == 0), stop=(k == MH - 1))
            nc.vector.tensor_copy(ybs[:, m, :], ps[:])
            if m in (3, 6, MD - 1):
                m0 = {3: 0, 6: 4, MD - 1: 7}[m]
                nc.sync.dma_start(
                    yT[:, m0:m + 1, C:].bitcast(dt.float16),
                    ybs[:, m0:m + 1, :])

    nc.compile()
    return nc


def _route(x, gate_w, expert_bias):
    """Exact numpy mirror of the reference TopKRouter + dispatch."""
    xf = x.reshape(-1, DIM).astype(np.float32)
    logits = xf @ gate_w.T.astype(np.float32)
    scores = 1.0 / (1.0 + np.exp(-logits.astype(np.float32)))
    biased = scores + expert_bias[None, :].astype(np.float32)
    # top-2, ties -> lower index (matches jax.lax.top_k)
    sel = np.argsort(-biased, axis=-1, kind="stable")[:, :TOPK]
    top_scores = np.take_along_axis(scores, sel, axis=-1)
    flat_sel = sel.reshape(-1)
    counts = np.bincount(flat_sel, minlength=E)
    order = np.argsort(flat_sel, kind="stable")
    scores_sorted = top_scores.reshape(-1)[order]
    token_ids = order // TOPK
    return xf, counts, order, token_ids, scores_sorted


def _kchunk(mat, width):
    """(n_k*128, width) row-major -> (128, n_k, width)."""
    return mat.reshape(-1, 128, width).transpose(1, 0, 2)


def _slabs(mat, n_m):
    """(n_k*128, n_m*128) weight matrix -> (128, n_m*n_k*128) m-major slabs
    of k-tiles, matching the device-side m-outer/k-inner consumption."""
    n_k = mat.shape[0] // 128
    kc = _kchunk(mat, mat.shape[1])                  # (128, n_k, n_m*128)
    return np.ascontiguousarray(
        kc.reshape(128, n_k, n_m, 128).transpose(0, 2, 1, 3).reshape(128, -1))


def kernel(x, gate_w, expert_bias, w_up, w_down, shared_w_up, shared_w_down):
    global LAST_RESULT
    from concourse.bass_utils import run_bass_kernel_spmd

    xf, counts, order, token_ids, scores_sorted = _route(x, gate_w, expert_bias)

    C = 512
    starts = np.zeros(E + 1, np.int64)
    np.cumsum(counts, out=starts[1:])

    # dispatch: routed_in rows grouped by expert, scaled by router score
    routed_in = (xf[token_ids] * scores_sorted[:, None]).astype(np.float32)
    routed_in = routed_in.astype(bf16)

    su_pack = _slabs(shared_w_up.T.astype(np.float16), MH)    # (128, 11264)
    sd_pack = _slabs(shared_w_down.T.astype(np.float16), MD)  # (128, 11264)
    in_maps = []
    for e in range(NCORES):
        seg = routed_in[starts[e]:starts[e] + min(int(counts[e]), C)]
        xrm = np.zeros((C, DIM), bf16)
        xrm[:seg.shape[0]] = seg
        xs_pack = _kchunk(
            xf[e * S:(e + 1) * S].T.astype(np.float16), S).reshape(128, -1)
        in_maps.append({
            "sT": np.ascontiguousarray(
                np.concatenate([xs_pack, su_pack], axis=1)),
            "rT": np.ascontiguousarray(np.concatenate(
                [_kchunk(xrm.T, C).reshape(128, -1),
                 _slabs(w_up[e].astype(bf16).T, MH)], axis=1)),
            "dT": _slabs(w_down[e].astype(bf16).T, MD),
            "eT": sd_pack,
        })

    key = (C, NWARM, NBRIDGE)
    if key not in _PROG_CACHE:
        _PROG_CACHE[key] = _build_program(C)
    nc = _PROG_CACHE[key]

    res = run_bass_kernel_spmd(
        nc, in_maps, list(range(NCORES)),
        trace=TRACE,
        trace_cores=TRACE_CORES,
        tmpdir=TRACE_DIR,
    )
    LAST_RESULT = res

    # --- combine (host): scatter per-expert outputs back to token order;
    # overflow slots beyond capacity get the exact fp32 expert MLP here ---
    routed_sorted = np.empty((T * TOPK, DIM), np.float32)
    for e in range(NCORES):
        arr = np.asarray(res.results[e]["yT"])             # (128, MD, C+S)
        yr = arr[:, :, :C].transpose(1, 0, 2).reshape(DIM, C).T
        n_dev = min(int(counts[e]), C)
        routed_sorted[starts[e]:starts[e] + n_dev] = \
            yr[:n_dev].astype(np.float32)
        if counts[e] > C:
            xo = routed_in[starts[e] + C:starts[e + 1]].astype(np.float32)
            ho = np.square(np.maximum(xo @ w_up[e].T.astype(np.float32), 0)
                           .astype(bf16).astype(np.float32)).astype(bf16)
            yo = ho.astype(np.float32) @ w_down[e].T.astype(np.float32)
            routed_sorted[starts[e] + C:starts[e + 1]] = \
                yo.astype(bf16).astype(np.float32)
    combined = np.empty((T * TOPK, DIM), np.float32)
    combined[order] = routed_sorted
    out = combined.reshape(T, TOPK, DIM).sum(axis=1)

    for e in range(NCORES):
        arr = np.asarray(res.results[e]["yT"])
        ys = np.ascontiguousarray(arr[:, :, C:]).view(np.float16)  # (128,MD,S)
        out[e * S:(e + 1) * S] += ys.transpose(1, 0, 2).reshape(DIM, S).T

    return out.reshape(1, T, DIM).astype(np.float32)

